# revision 1
# baseline (speedup 1.0000x reference)
"""CRF loss (2-state FSA) on 8 Trainium2 NeuronCores.

Math: with y = exp(log_probs), the per-step denominator scores are linear in y:
  E0 = log S0, S0 = sum_c y[c]*U0[c];  E1 = log S1, S1 = sum_c y[c]*U1[c]
where U0/U1 are softmax segments of den_scores mapped through the arc table.
The 2-state forward recurrence runs in REAL space as products of 2x2 matrices
  M_t = [[S0, S1], [p*e00, p*e11]],  p = y[2] = exp(lp[:, 2])
composed on-device over chunks of L=8 steps (scaled by 32 per step to avoid
underflow; exact correction 8*ln(32) removed on host). Steps past input_len
become 32*I (identity under uniform scaling). The host folds per-sequence
chunk matrices in log space (gather/unshard-scale work) and sums partials.

Numerator: per-position gather lp[bt, label] == ln(y16[bt, label]) extracted
with tensor_mask_reduce (per-partition [label, label+1) range mask + max).

Sharding: data-parallel over batch; core k owns sequences [8k, 8k+8).
Per-core layout: partition p holds 256 consecutive (b, t) rows; sequence of a
partition = p // 16, t-offset = (p % 16) * 256 (fully contiguous DMA loads).
"""

import os
import sys

import numpy as np

for _p in ("/opt/trn_rl_repo", os.path.expanduser("~/.axon_site/_ro/trn_rl_repo")):
    if os.path.isdir(_p) and _p not in sys.path:
        sys.path.insert(0, _p)

import concourse.bacc as bacc
import concourse.bass as bass
import concourse.mybir as mybir
import concourse.tile as tile
from concourse.bass_utils import run_bass_kernel_spmd

F32 = mybir.dt.float32
BF16 = mybir.dt.bfloat16
I32 = mybir.dt.int32
Alu = mybir.AluOpType
Act = mybir.ActivationFunctionType

L = 125
C = 128          # symbol classes
B, T = 64, 4096
NCORES = 8
BSH = B // NCORES            # sequences per core = 8
BT = BSH * T                 # positions per core = 32768
NI = BT // 128               # free positions per partition = 256
NQ = 4                       # quarters (DMA/compute pipelining)
NIQ = NI // NQ               # 64 positions per quarter
LCH = 2                      # scan chunk length (steps composed on device)
NCH = NI // LCH              # 128 chunk matrices per partition
SCALE = 32.0                 # per-step scaling against fp32 underflow
NEGBIG = -3.0e38


def _build_program():
    nc = bacc.Bacc("TRN2", target_bir_lowering=False, debug=False)

    lp_d = nc.dram_tensor("lp", [BT, C], F32, kind="ExternalInput")
    lab_d = nc.dram_tensor("lab", [128, NI], I32, kind="ExternalInput")
    lens_d = nc.dram_tensor("lens", [BSH, 1], F32, kind="ExternalInput")
    den_d = nc.dram_tensor("den2", [2, C], F32, kind="ExternalInput")
    iota_d = nc.dram_tensor("iota_i", [128, NI], F32, kind="ExternalInput")
    iotac_d = nc.dram_tensor("iota_c", [128, C], F32, kind="ExternalInput")
    offs_d = nc.dram_tensor("offs", [128, 1], F32, kind="ExternalInput")
    e8_d = nc.dram_tensor("e8", [BSH, 128], F32, kind="ExternalInput")
    i2_d = nc.dram_tensor("i2", [2, 2], F32, kind="ExternalInput")
    sel_d = nc.dram_tensor("sel01", [2, 2, 128], F32, kind="ExternalInput")

    chunklog_d = nc.dram_tensor("chunklog", [128, 4, NCH], F32, kind="ExternalOutput")
    numpart_d = nc.dram_tensor("numpart", [128, 1], F32, kind="ExternalOutput")
    fs_d = nc.dram_tensor("fs", [1, 1], F32, kind="ExternalOutput")

    with tile.TileContext(nc) as tc:
        with (
            tc.tile_pool(name="const", bufs=1) as cpool,
            tc.tile_pool(name="small", bufs=1) as spool,
            tc.tile_pool(name="lp16", bufs=2) as lp_pool,
            tc.tile_pool(name="y16", bufs=2) as y_pool,
            tc.tile_pool(name="yT", bufs=1) as yT_pool,
            tc.tile_pool(name="tmr", bufs=2) as tmr_pool,
            tc.tile_pool(name="scan", bufs=1) as scan_pool,
            tc.tile_pool(name="psS", bufs=4, space=bass.MemorySpace.PSUM) as psS,
            tc.tile_pool(name="psU", bufs=1, space=bass.MemorySpace.PSUM) as psU,
        ):
            # ---------------- constants ----------------
            iota_i = cpool.tile([128, NI], F32)
            nc.sync.dma_start(iota_i[:], iota_d.ap())
            iota_c = cpool.tile([128, C], F32)
            nc.sync.dma_start(iota_c[:], iotac_d.ap())
            offs = cpool.tile([128, 1], F32)
            nc.sync.dma_start(offs[:], offs_d.ap())
            e8 = cpool.tile([BSH, 128], F32)
            nc.sync.dma_start(e8[:], e8_d.ap())
            i2 = cpool.tile([2, 2], F32)
            nc.sync.dma_start(i2[:], i2_d.ap())
            sel0 = cpool.tile([2, 128], F32)
            nc.sync.dma_start(sel0[:], sel_d.ap()[0:1, :, :].rearrange("a k m -> (a k) m"))
            sel1 = cpool.tile([2, 128], F32)
            nc.sync.dma_start(sel1[:], sel_d.ap()[1:2, :, :].rearrange("a k m -> (a k) m"))
            den_sb = cpool.tile([2, C], F32)
            nc.sync.dma_start(den_sb[:], den_d.ap())
            lens_sb = cpool.tile([BSH, 1], F32)
            nc.sync.dma_start(lens_sb[:], lens_d.ap())
            lab_sb = cpool.tile([128, NI], I32)
            nc.sync.dma_start(lab_sb[:], lab_d.ap())

            # ---------------- arc weights (u = per-state softmax) ----------------
            ed = spool.tile([2, C], F32)
            nc.scalar.activation(ed[:], den_sb[:], Act.Exp)
            zd = spool.tile([2, 1], F32)
            nc.vector.tensor_reduce(zd[:], ed[:], mybir.AxisListType.X, Alu.add)
            rd = spool.tile([2, 1], F32)
            nc.vector.reciprocal(rd[:], zd[:])
            u = spool.tile([2, C], F32)
            nc.vector.tensor_scalar(u[:], ed[:], rd[:, 0:1], None, Alu.mult)

            # urows[0, c] = E0 weight of class c, urows[1, c] = E1 weight
            urows = spool.tile([2, C], F32)
            nc.gpsimd.memset(urows[:], 0.0)
            nc.vector.tensor_copy(urows[0:1, 1:2], u[0:1, 0:1])
            nc.vector.tensor_copy(urows[0:2, 3:128], u[0:2, 1:126])
            u_ps = psU.tile([128, 2], F32)
            nc.tensor.matmul(u_ps[:], urows[:], i2[:], start=True, stop=True)
            # split-bf16 weights: U = hi + lo keeps effective fp32 precision
            # through the bf16 matmul (two PSUM-accumulated matmuls per block)
            uf = spool.tile([128, 2], F32)
            nc.vector.tensor_copy(uf[:], u_ps[:])
            u16 = spool.tile([128, 4], BF16)
            nc.vector.tensor_copy(u16[:, 0:2], uf[:])
            ulo = spool.tile([128, 2], F32)
            nc.vector.tensor_sub(ulo[:], uf[:], u16[:, 0:2])
            nc.vector.tensor_copy(u16[:, 2:4], ulo[:])

            # e_b[:, 0] = e00 = u0[126] bcast, e_b[:, 1] = e11 = u1[0] bcast
            # (selector matmuls: sel0/sel1 pick row 0 / row 1 of u)
            e_ps = psU.tile([128, 2], F32)
            nc.tensor.matmul(
                e_ps[:, 0:1], sel0[:], u[0:2, 126:127], start=True, stop=True
            )
            nc.tensor.matmul(
                e_ps[:, 1:2], sel1[:], u[0:2, 0:1], start=True, stop=True
            )
            e_b = spool.tile([128, 2], F32)
            nc.vector.tensor_copy(e_b[:], e_ps[:])

            # final arc score (log u0[127]) -> output
            fs_sb = spool.tile([1, 1], F32)
            nc.scalar.activation(fs_sb[:], u[0:1, 127:128], Act.Ln)
            nc.sync.dma_start(fs_d.ap(), fs_sb[:])

            # ---------------- per-partition length mask ----------------
            thr_ps = psU.tile([128, 1], F32)
            nc.tensor.matmul(thr_ps[:], e8[:], lens_sb[:], start=True, stop=True)
            thr = spool.tile([128, 1], F32)
            nc.vector.tensor_tensor(thr[:], thr_ps[:], offs[:], Alu.subtract)
            m01 = spool.tile([128, NI], F32)
            nc.vector.tensor_scalar(m01[:], iota_i[:], thr[:, 0:1], None, Alu.is_lt)
            w32 = spool.tile([128, NI], F32)   # 32*m
            nc.vector.tensor_scalar(w32[:], m01[:], SCALE, None, Alu.mult)
            w32c = spool.tile([128, NI], F32)  # 32*(1-m)
            nc.vector.tensor_scalar(w32c[:], m01[:], -SCALE, SCALE, Alu.mult, Alu.add)

            # labels with validity folded in: invalid positions -> 200 (no
            # class matches, so masked positions contribute exactly 0)
            labf = spool.tile([128, NI], F32)
            nc.vector.tensor_copy(labf[:], lab_sb[:])
            nc.vector.tensor_scalar(labf[:], labf[:], -200.0, None, Alu.add)
            nc.vector.tensor_tensor(labf[:], labf[:], m01[:], Alu.mult)
            nc.vector.tensor_scalar(labf[:], labf[:], 200.0, None, Alu.add)

            # ---------------- persistent main buffers ----------------
            s01 = spool.tile([128, 2 * NI], F32)    # S0/S1 interleaved by position
            p_buf = spool.tile([128, NI], F32)      # y[:, 2] per position
            numq = spool.tile([128, 16], F32)        # per-group numerator sums
            yT = yT_pool.tile([128, BT], BF16)      # transposed exp, block-major

            lp_view = lp_d.ap().rearrange("(p i) c -> p i c", p=128)

            # ---------------- streaming main loop ----------------
            for q in range(NQ):
                lp32 = lp_pool.tile([128, NIQ * C], F32)
                nc.sync.dma_start(
                    lp32[:], lp_view[:, q * NIQ : (q + 1) * NIQ, :]
                )
                y16 = y_pool.tile([128, NIQ * C], BF16)
                nc.scalar.activation(y16[:], lp32[:], Act.Exp)

                y3 = y16[:].rearrange("p (i c) -> p i c", c=C)
                nc.scalar.copy(
                    p_buf[:, q * NIQ : (q + 1) * NIQ], y3[:, :, 2:3]
                )

                yT_q = yT[:, q * NIQ * C : (q + 1) * NIQ * C].rearrange(
                    "p (j z) -> p j z", z=128
                )
                nc.sync.dma_start_transpose(yT_q, y16[:])

                for g in range(NIQ // 8):
                    ps = psS.tile([128, 16], F32)
                    for jj in range(8):
                        j = g * 8 + jj
                        blk = yT[
                            :, (q * NIQ + j) * 128 : (q * NIQ + j + 1) * 128
                        ]
                        nc.tensor.matmul(
                            ps[:, 2 * jj : 2 * jj + 2], blk, u16[:, 0:2],
                            start=True, stop=False,
                        )
                        nc.tensor.matmul(
                            ps[:, 2 * jj : 2 * jj + 2], blk, u16[:, 2:4],
                            start=False, stop=True,
                        )
                    i0 = q * NIQ + g * 8
                    nc.scalar.copy(s01[:, 2 * i0 : 2 * i0 + 16], ps[:])

                # numerator gather: (iota_c == label) * lp, one nonzero term
                # per position (exact fp32). DVE accum_out crashes the device
                # on this stack, so the group sums run on the scalar engine
                # (ACT accum_out — HW-verified safe); invalid positions were
                # already zeroed via the label-folding above.
                GRP = 16
                for h in range(NIQ // GRP):
                    mout = tmr_pool.tile([128, GRP * C], F32)
                    for j in range(GRP):
                        jq = h * GRP + j
                        i = q * NIQ + jq
                        nc.vector.scalar_tensor_tensor(
                            mout[:, j * C : (j + 1) * C],
                            iota_c[:],
                            labf[:, i : i + 1],
                            lp32[:, jq * C : (jq + 1) * C],
                            Alu.is_equal,
                            Alu.mult,
                        )
                    scr = tmr_pool.tile([128, GRP * C], BF16, tag="scr")
                    nc.scalar.activation(
                        scr[:], mout[:], Act.Identity,
                        accum_out=numq[:, 4 * q + h : 4 * q + h + 1],
                    )

            # ---------------- numerator ----------------
            numpart = spool.tile([128, 1], F32)
            nc.vector.tensor_reduce(
                numpart[:], numq[:], mybir.AxisListType.X, Alu.add
            )
            nc.sync.dma_start(numpart_d.ap(), numpart[:])

            # ---------------- scan step tensors (masked + scaled) ----------------
            s0v = s01[:].rearrange("p (i two) -> p i two", two=2)[:, :, 0:1]
            s1v = s01[:].rearrange("p (i two) -> p i two", two=2)[:, :, 1:2]
            sm0 = spool.tile([128, NI], F32)
            nc.gpsimd.tensor_tensor(sm0[:], s0v, w32[:], Alu.mult)
            nc.gpsimd.tensor_tensor(sm0[:], sm0[:], w32c[:], Alu.add)
            sm1 = spool.tile([128, NI], F32)
            nc.gpsimd.tensor_tensor(sm1[:], s1v, w32[:], Alu.mult)
            a_t = spool.tile([128, NI], F32)
            nc.gpsimd.tensor_scalar(a_t[:], p_buf[:], e_b[:, 0:1], None, Alu.mult)
            nc.gpsimd.tensor_tensor(a_t[:], a_t[:], w32[:], Alu.mult)
            b_t = spool.tile([128, NI], F32)
            nc.gpsimd.tensor_scalar(b_t[:], p_buf[:], e_b[:, 1:2], None, Alu.mult)
            nc.gpsimd.tensor_tensor(b_t[:], b_t[:], w32[:], Alu.mult)
            nc.gpsimd.tensor_tensor(b_t[:], b_t[:], w32c[:], Alu.add)

            def step_slice(tile_ap, t):
                return tile_ap[:].rearrange("p (c l) -> p c l", l=LCH)[:, :, t : t + 1]

            # ---------------- on-device chunk scan (real space) ----------------
            P = {}
            for name, src in (("00", sm0), ("01", sm1), ("10", a_t), ("11", b_t)):
                pt = scan_pool.tile([128, NCH], F32, tag=f"P{name}")
                nc.scalar.copy(pt[:], step_slice(src, 0))
                P[name] = pt

            for t in range(1, LCH):
                s0t, s1t = step_slice(sm0, t), step_slice(sm1, t)
                att, btt = step_slice(a_t, t), step_slice(b_t, t)
                newP = {}
                for col in ("0", "1"):
                    pc0, pc1 = P["0" + col], P["1" + col]
                    n0 = scan_pool.tile([128, NCH], F32, tag=f"n0{col}")
                    t1 = scan_pool.tile([128, NCH], F32, tag="t1")
                    nc.gpsimd.tensor_tensor(t1[:], s0t, pc0[:], Alu.mult)
                    nc.gpsimd.tensor_tensor(n0[:], s1t, pc1[:], Alu.mult)
                    nc.gpsimd.tensor_tensor(n0[:], t1[:], n0[:], Alu.add)
                    n1 = scan_pool.tile([128, NCH], F32, tag=f"n1{col}")
                    t2 = scan_pool.tile([128, NCH], F32, tag="t2")
                    nc.gpsimd.tensor_tensor(t2[:], att, pc0[:], Alu.mult)
                    nc.gpsimd.tensor_tensor(n1[:], btt, pc1[:], Alu.mult)
                    nc.gpsimd.tensor_tensor(n1[:], t2[:], n1[:], Alu.add)
                    newP["0" + col] = n0
                    newP["1" + col] = n1
                P = newP

            for e, name in enumerate(("00", "01", "10", "11")):
                plog = scan_pool.tile([128, NCH], F32, tag="plog")
                nc.scalar.activation(plog[:], P[name][:], Act.Ln)
                nc.sync.dma_start(chunklog_d.ap()[:, e : e + 1, :], plog[:])

    nc.compile()
    return nc


_NC_CACHE = None


def _get_program():
    global _NC_CACHE
    if _NC_CACHE is None:
        _NC_CACHE = _build_program()
    return _NC_CACHE


def _make_in_maps(log_probs, den_scores, input_lens, labels):
    pids = np.arange(128)
    iota_i = np.broadcast_to(
        np.arange(NI, dtype=np.float32), (128, NI)
    ).copy()
    iota_c = np.broadcast_to(
        np.arange(C, dtype=np.float32), (128, C)
    ).copy()
    offs = ((pids % 16) * NI).astype(np.float32).reshape(128, 1)
    e8 = (pids[None, :] // 16 == np.arange(BSH)[:, None]).astype(np.float32)
    i2 = np.eye(2, dtype=np.float32)
    sel01 = np.zeros((2, 2, 128), dtype=np.float32)
    sel01[0, 0, :] = 1.0
    sel01[1, 1, :] = 1.0
    den2 = np.full((2, C), -1e30, dtype=np.float32)
    den2[0, :] = den_scores[: L + 3]
    den2[1, : L + 1] = den_scores[L + 3 :]

    in_maps = []
    for k in range(NCORES):
        sl = slice(k * BSH, (k + 1) * BSH)
        in_maps.append(
            dict(
                lp=np.ascontiguousarray(
                    log_probs[sl].reshape(BT, C), dtype=np.float32
                ),
                lab=np.ascontiguousarray(
                    labels[sl].reshape(128, NI), dtype=np.int32
                ),
                lens=input_lens[sl].astype(np.float32).reshape(BSH, 1),
                den2=den2,
                iota_i=iota_i,
                iota_c=iota_c,
                offs=offs,
                e8=e8,
                i2=i2,
                sel01=sel01,
            )
        )
    return in_maps


def _combine_host(results):
    """Fold per-core device outputs into the scalar loss (float64 host fold)."""
    num = 0.0
    logM_all = []  # [64, NCHUNKS_TOTAL, 2, 2] in global sequence order
    fs = None
    corr = LCH * np.log(SCALE)
    for res in results:
        num += float(res["numpart"].sum(dtype=np.float64))
        fs = float(res["fs"][0, 0])
        cl = res["chunklog"].astype(np.float64)  # [128, 4, NCH]
        # partition p -> (seq_local = p//16, toff = p%16); chunk order (toff, c)
        cl = cl.reshape(BSH, 16, 4, NCH)
        cl = np.transpose(cl, (0, 1, 3, 2)).reshape(BSH, 16 * NCH, 2, 2)
        logM_all.append(cl - corr)
    mats = np.concatenate(logM_all, axis=0)  # [64, 512, 2, 2]

    def compose(Bm, Am):
        # C = B o A : C[i,j] = LSE_k(B[i,k] + A[k,j])
        s = Bm[..., :, :, None] + Am[..., None, :, :]  # [..., i, k, j]
        return _lse(s, axis=-2)

    while mats.shape[1] > 1:
        n = mats.shape[1]
        if n % 2:
            last = mats[:, -1:]
            mats = compose(mats[:, 1::2], mats[:, 0:-1:2])
            mats = np.concatenate([mats, last], axis=1)
        else:
            mats = compose(mats[:, 1::2], mats[:, 0::2])
    den = float(mats[:, 0, 0, 0].sum()) + B * fs
    return np.float32(num - den)


def _lse(x, axis):
    m = np.max(x, axis=axis, keepdims=True)
    m = np.where(np.isfinite(m), m, 0.0)
    out = np.squeeze(m, axis) + np.log(
        np.sum(np.exp(x - m), axis=axis)
    )
    return out


def kernel(log_probs, den_scores, input_lens, labels):
    nc = _get_program()
    in_maps = _make_in_maps(
        np.asarray(log_probs), np.asarray(den_scores),
        np.asarray(input_lens), np.asarray(labels),
    )
    res = run_bass_kernel_spmd(nc, in_maps, core_ids=list(range(NCORES)))
    return _combine_host(res.results)



# revision 5
# speedup vs baseline: 1.5538x; 1.5538x over previous
"""CRF loss (2-state FSA) on 8 Trainium2 NeuronCores — transposed-layout v2.

Math: with y = exp(log_probs), the per-step denominator scores are linear in y:
  S0 = sum_c y[c]*U0[c];  S1 = sum_c y[c]*U1[c];  p = y[2]
where U0/U1 are softmax segments of den_scores mapped through the arc table.
The 2-state forward recurrence runs in REAL space as products of 2x2 matrices
  M_t = [[S0, S1], [p*e00, p*e11]]
composed on-device over chunks of L=2 steps (scaled by 32 per step against
underflow; exact correction removed on host). Steps past input_len become
32*I. The host folds per-sequence chunk matrices in log space and sums.

Layout: host ships lp TRANSPOSED as lpT[c, g] bf16 with column order
g = i*128 + pi, where pi = (seq_local*16 + toff) is the scan partition and
i the within-partition step (t = toff*256 + i). Then:
  - exp on ACT gives y16T in the same layout;
  - S0/S1/p for scan column i = one tiny PE matmul y16T[:, i-block].T @ W
    (W columns: U0hi, U1hi, onehot(2), U0lo, U1lo — hi/lo bf16 split keeps
    fp32-level U precision) accumulating straight into scan-layout PSUM;
  - numerator: PE broadcasts labels (ones[1,128].T @ labT) into PSUM, one
    fused DVE STT per chunk computes (lab==iota_c)*lpT, and PE ones-matmuls
    accumulate the masked values into a [128,1] PSUM (exact bf16 gather).

Sharding: data-parallel over batch; core k owns sequences [8k, 8k+8).
"""

import os
import sys

import numpy as np
import ml_dtypes

for _p in ("/opt/trn_rl_repo", os.path.expanduser("~/.axon_site/_ro/trn_rl_repo")):
    if os.path.isdir(_p) and _p not in sys.path:
        sys.path.insert(0, _p)

import concourse.bacc as bacc
import concourse.bass as bass
import concourse.mybir as mybir
import concourse.tile as tile
from concourse.bass_utils import run_bass_kernel_spmd

F32 = mybir.dt.float32
BF16 = mybir.dt.bfloat16
I32 = mybir.dt.int32
Alu = mybir.AluOpType
Act = mybir.ActivationFunctionType
Bfollowing = None

L = 125
C = 128          # symbol classes
B, T = 64, 4096
NCORES = 8
BSH = B // NCORES            # sequences per core = 8
BT = BSH * T                 # positions per core = 32768
NI = BT // 128               # steps per scan partition = 256
NQ = 4                       # quarters (DMA/compute pipelining)
GQ = BT // NQ                # positions per quarter = 8192
NIQ = NI // NQ               # scan steps per quarter = 64
LCH = 2                      # scan chunk length (steps composed on device)
NCH = NI // LCH              # 128 chunk matrices per partition
SCALE = 32.0                 # per-step scaling against fp32 underflow
NCK = 8                      # numerator STT chunks per quarter
CK = GQ // NCK               # chunk positions = 1024


def _build_program():
    nc = bacc.Bacc("TRN2", target_bir_lowering=False, debug=False)

    lpt_d = nc.dram_tensor("lpt", [128, BT], BF16, kind="ExternalInput")
    labt_d = nc.dram_tensor("labt", [1, BT], BF16, kind="ExternalInput")
    lens_d = nc.dram_tensor("lens", [BSH, 1], F32, kind="ExternalInput")
    den_d = nc.dram_tensor("den2", [2, C], F32, kind="ExternalInput")
    iota_d = nc.dram_tensor("iota_i", [128, NI], F32, kind="ExternalInput")
    iotac_d = nc.dram_tensor("iota_c", [128, 1], F32, kind="ExternalInput")
    offs_d = nc.dram_tensor("offs", [128, 1], F32, kind="ExternalInput")
    e8_d = nc.dram_tensor("e8", [BSH, 128], F32, kind="ExternalInput")
    i2_d = nc.dram_tensor("i2", [2, 2], F32, kind="ExternalInput")
    sel_d = nc.dram_tensor("sel01", [2, 2, 128], F32, kind="ExternalInput")
    e2_d = nc.dram_tensor("e2", [128, 1], F32, kind="ExternalInput")
    ones_d = nc.dram_tensor("ones1", [1, 128], F32, kind="ExternalInput")

    chunklog_d = nc.dram_tensor("chunklog", [128, 4, NCH], F32, kind="ExternalOutput")
    numpart_d = nc.dram_tensor("numpart", [128, 1], F32, kind="ExternalOutput")
    fs_d = nc.dram_tensor("fs", [1, 1], F32, kind="ExternalOutput")

    with tile.TileContext(nc) as tc:
        with (
            tc.tile_pool(name="const", bufs=1) as cpool,
            tc.tile_pool(name="small", bufs=1) as spool,
            tc.tile_pool(name="lpt", bufs=2) as lp_pool,
            tc.tile_pool(name="y16", bufs=2) as y_pool,
            tc.tile_pool(name="msk", bufs=2) as m_pool,
            tc.tile_pool(name="scan", bufs=1) as scan_pool,
            tc.tile_pool(name="psS", bufs=2, space=bass.MemorySpace.PSUM) as psS,
            tc.tile_pool(name="psL", bufs=2, space=bass.MemorySpace.PSUM) as psL,
            tc.tile_pool(name="psN", bufs=1, space=bass.MemorySpace.PSUM) as psN,
            tc.tile_pool(name="psU", bufs=1, space=bass.MemorySpace.PSUM) as psU,
        ):
            # ---------------- constants ----------------
            iota_i = cpool.tile([128, NI], F32)
            nc.sync.dma_start(iota_i[:], iota_d.ap())
            iota_c = cpool.tile([128, 1], F32)
            nc.sync.dma_start(iota_c[:], iotac_d.ap())
            offs = cpool.tile([128, 1], F32)
            nc.sync.dma_start(offs[:], offs_d.ap())
            e8 = cpool.tile([BSH, 128], F32)
            nc.sync.dma_start(e8[:], e8_d.ap())
            i2 = cpool.tile([2, 2], F32)
            nc.sync.dma_start(i2[:], i2_d.ap())
            sel0 = cpool.tile([2, 128], F32)
            nc.sync.dma_start(sel0[:], sel_d.ap()[0:1, :, :].rearrange("a k m -> (a k) m"))
            sel1 = cpool.tile([2, 128], F32)
            nc.sync.dma_start(sel1[:], sel_d.ap()[1:2, :, :].rearrange("a k m -> (a k) m"))
            den_sb = cpool.tile([2, C], F32)
            nc.sync.dma_start(den_sb[:], den_d.ap())
            lens_sb = cpool.tile([BSH, 1], F32)
            nc.sync.dma_start(lens_sb[:], lens_d.ap())
            e2f = cpool.tile([128, 1], F32)
            nc.sync.dma_start(e2f[:], e2_d.ap())
            ones1 = cpool.tile([1, 128], BF16)
            onesf = cpool.tile([1, 128], F32)
            nc.sync.dma_start(onesf[:], ones_d.ap())
            nc.vector.tensor_copy(ones1[:], onesf[:])
            ones128 = cpool.tile([128, 1], BF16)
            nc.gpsimd.memset(ones128[:], 1.0)
            labt_sb = cpool.tile([1, BT], BF16)
            nc.sync.dma_start(labt_sb[:], labt_d.ap())

            # ---------------- arc weights (u = per-state softmax) ----------------
            ed = spool.tile([2, C], F32)
            nc.scalar.activation(ed[:], den_sb[:], Act.Exp)
            zd = spool.tile([2, 1], F32)
            nc.vector.tensor_reduce(zd[:], ed[:], mybir.AxisListType.X, Alu.add)
            rd = spool.tile([2, 1], F32)
            nc.vector.reciprocal(rd[:], zd[:])
            u = spool.tile([2, C], F32)
            nc.vector.tensor_scalar(u[:], ed[:], rd[:, 0:1], None, Alu.mult)

            # urows[0, c] = U0 weight of class c, urows[1, c] = U1 weight
            urows = spool.tile([2, C], F32)
            nc.gpsimd.memset(urows[:], 0.0)
            nc.vector.tensor_copy(urows[0:1, 1:2], u[0:1, 0:1])
            nc.vector.tensor_copy(urows[0:2, 3:128], u[0:2, 1:126])
            u_ps = psU.tile([128, 2], F32, tag="init")
            nc.tensor.matmul(u_ps[:], urows[:], i2[:], start=True, stop=True)
            uf = spool.tile([128, 2], F32)
            nc.vector.tensor_copy(uf[:], u_ps[:])
            # moving operand W: [U0hi, U1hi, onehot(2), U0lo, U1lo, 0, 0, 0]
            # (hi+lo bf16 split keeps effective fp32 U through the matmul)
            w8 = spool.tile([128, 8], BF16)
            nc.gpsimd.memset(w8[:], 0.0)
            nc.vector.tensor_copy(w8[:, 0:2], uf[:])
            ulo = spool.tile([128, 2], F32)
            nc.vector.tensor_sub(ulo[:], uf[:], w8[:, 0:2])
            nc.vector.tensor_copy(w8[:, 3:5], ulo[:])
            nc.vector.tensor_copy(w8[:, 2:3], e2f[:])

            # e_b[:, 0] = e00 = u0[126] bcast, e_b[:, 1] = e11 = u1[0] bcast
            e_ps = psU.tile([128, 2], F32, tag="init")
            nc.tensor.matmul(
                e_ps[:, 0:1], sel0[:], u[0:2, 126:127], start=True, stop=True
            )
            nc.tensor.matmul(
                e_ps[:, 1:2], sel1[:], u[0:2, 0:1], start=True, stop=True
            )
            e_b = spool.tile([128, 2], F32)
            nc.vector.tensor_copy(e_b[:], e_ps[:])

            # ---------------- per-partition length mask ----------------
            thr_ps = psU.tile([128, 1], F32, tag="init")
            nc.tensor.matmul(thr_ps[:], e8[:], lens_sb[:], start=True, stop=True)
            thr = spool.tile([128, 1], F32)
            nc.vector.tensor_tensor(thr[:], thr_ps[:], offs[:], Alu.subtract)
            m01 = spool.tile([128, NI], F32)
            nc.vector.tensor_scalar(m01[:], iota_i[:], thr[:, 0:1], None, Alu.is_lt)
            w32 = spool.tile([128, NI], F32)   # 32*m
            nc.vector.tensor_scalar(w32[:], m01[:], SCALE, None, Alu.mult)
            w32c = spool.tile([128, NI], F32)  # 32*(1-m)
            nc.vector.tensor_scalar(w32c[:], m01[:], -SCALE, SCALE, Alu.mult, Alu.add)

            # ---------------- persistent scan-quantity buffer ----------------
            # v_sb[pi, 8*i + n]: n=0,1 U-hi S0/S1; n=2 p; n=3,4 U-lo S0/S1
            v_sb = spool.tile([128, 8 * NI], F32)

            numacc = psN.tile([128, 1], F32)
            n_mm = 0
            N_MM_TOTAL = NQ * NCK * (CK // 128)

            # ---------------- streaming main loop ----------------
            for q in range(NQ):
                lpt = lp_pool.tile([128, GQ], BF16)
                nc.sync.dma_start(lpt[:], lpt_d.ap()[:, q * GQ : (q + 1) * GQ])
                y16 = y_pool.tile([128, GQ], BF16)
                nc.scalar.activation(y16[:], lpt[:], Act.Exp)

                # S0/S1/p for the quarter's 64 scan steps -> scan-layout PSUM
                s_ps = psS.tile([128, 8 * NIQ], F32)
                for j in range(NIQ):
                    nc.tensor.matmul(
                        s_ps[:, 8 * j : 8 * j + 8],
                        y16[:, j * 128 : (j + 1) * 128],
                        w8[:],
                        start=True, stop=True,
                    )
                nc.vector.tensor_copy(
                    v_sb[:, q * 8 * NIQ : (q + 1) * 8 * NIQ], s_ps[:]
                )

                # numerator: broadcast labels via PE, mask+extract via STT,
                # reduce via PE ones-matmuls into numacc
                for h in range(NCK):
                    lab_ps = psL.tile([128, CK], F32)
                    for z in range(CK // 512):
                        nc.tensor.matmul(
                            lab_ps[:, z * 512 : (z + 1) * 512],
                            ones1[:],
                            labt_sb[:, q * GQ + h * CK + z * 512 :
                                    q * GQ + h * CK + (z + 1) * 512],
                            start=True, stop=True,
                        )
                    msk = m_pool.tile([128, CK], BF16)
                    nc.vector.scalar_tensor_tensor(
                        msk[:],
                        lab_ps[:],
                        iota_c[:, 0:1],
                        lpt[:, h * CK : (h + 1) * CK],
                        Alu.is_equal,
                        Alu.mult,
                    )
                    for z in range(CK // 128):
                        nc.tensor.matmul(
                            numacc[:],
                            msk[:, z * 128 : (z + 1) * 128],
                            ones128[:],
                            start=(n_mm == 0), stop=(n_mm == N_MM_TOTAL - 1),
                            skip_group_check=True,
                        )
                        n_mm += 1

            # ---------------- numerator out ----------------
            numpart = spool.tile([128, 1], F32)
            nc.vector.tensor_copy(numpart[:], numacc[:])
            nc.sync.dma_start(numpart_d.ap(), numpart[:])

            # final arc score (log u0[127]) -> output (here so the act-table
            # Exp->Ln swap happens once, after the streaming Exp passes)
            fs_sb = spool.tile([1, 1], F32)
            nc.scalar.activation(fs_sb[:], u[0:1, 127:128], Act.Ln)
            nc.sync.dma_start(fs_d.ap(), fs_sb[:])

            # ---------------- scan step tensors (masked + scaled) ----------------
            v3 = v_sb[:].rearrange("p (i n) -> p i n", n=8)
            s0r = spool.tile([128, NI], F32)
            nc.gpsimd.tensor_tensor(s0r[:], v3[:, :, 0:1], v3[:, :, 3:4], Alu.add)
            s1r = spool.tile([128, NI], F32)
            nc.gpsimd.tensor_tensor(s1r[:], v3[:, :, 1:2], v3[:, :, 4:5], Alu.add)

            sm0 = spool.tile([128, NI], F32)
            nc.gpsimd.tensor_tensor(sm0[:], s0r[:], w32[:], Alu.mult)
            nc.gpsimd.tensor_tensor(sm0[:], sm0[:], w32c[:], Alu.add)
            sm1 = spool.tile([128, NI], F32)
            nc.gpsimd.tensor_tensor(sm1[:], s1r[:], w32[:], Alu.mult)
            a_t = spool.tile([128, NI], F32)
            nc.gpsimd.tensor_scalar(a_t[:], v3[:, :, 2:3], e_b[:, 0:1], None, Alu.mult)
            nc.gpsimd.tensor_tensor(a_t[:], a_t[:], w32[:], Alu.mult)
            b_t = spool.tile([128, NI], F32)
            nc.gpsimd.tensor_scalar(b_t[:], v3[:, :, 2:3], e_b[:, 1:2], None, Alu.mult)
            nc.gpsimd.tensor_tensor(b_t[:], b_t[:], w32[:], Alu.mult)
            nc.gpsimd.tensor_tensor(b_t[:], b_t[:], w32c[:], Alu.add)

            def step_slice(tile_ap, t):
                return tile_ap[:].rearrange("p (c l) -> p c l", l=LCH)[:, :, t : t + 1]

            # ---------------- on-device chunk scan (real space) ----------------
            P = {}
            for name, src in (("00", sm0), ("01", sm1), ("10", a_t), ("11", b_t)):
                pt = scan_pool.tile([128, NCH], F32, tag=f"P{name}")
                nc.scalar.copy(pt[:], step_slice(src, 0))
                P[name] = pt

            for t in range(1, LCH):
                s0t, s1t = step_slice(sm0, t), step_slice(sm1, t)
                att, btt = step_slice(a_t, t), step_slice(b_t, t)
                newP = {}
                for col in ("0", "1"):
                    pc0, pc1 = P["0" + col], P["1" + col]
                    n0 = scan_pool.tile([128, NCH], F32, tag=f"n0{col}")
                    t1 = scan_pool.tile([128, NCH], F32, tag="t1")
                    nc.gpsimd.tensor_tensor(t1[:], s0t, pc0[:], Alu.mult)
                    nc.gpsimd.tensor_tensor(n0[:], s1t, pc1[:], Alu.mult)
                    nc.gpsimd.tensor_tensor(n0[:], t1[:], n0[:], Alu.add)
                    n1 = scan_pool.tile([128, NCH], F32, tag=f"n1{col}")
                    t2 = scan_pool.tile([128, NCH], F32, tag="t2")
                    nc.gpsimd.tensor_tensor(t2[:], att, pc0[:], Alu.mult)
                    nc.gpsimd.tensor_tensor(n1[:], btt, pc1[:], Alu.mult)
                    nc.gpsimd.tensor_tensor(n1[:], t2[:], n1[:], Alu.add)
                    newP["0" + col] = n0
                    newP["1" + col] = n1
                P = newP

            for e, name in enumerate(("00", "01", "10", "11")):
                plog = scan_pool.tile([128, NCH], F32, tag="plog")
                nc.scalar.activation(plog[:], P[name][:], Act.Ln)
                nc.sync.dma_start(chunklog_d.ap()[:, e : e + 1, :], plog[:])

    nc.compile()
    return nc


_NC_CACHE = None


def _get_program():
    global _NC_CACHE
    if _NC_CACHE is None:
        _NC_CACHE = _build_program()
    return _NC_CACHE


def _make_in_maps(log_probs, den_scores, input_lens, labels):
    bf16 = ml_dtypes.bfloat16
    pids = np.arange(128)
    iota_i = np.broadcast_to(
        np.arange(NI, dtype=np.float32), (128, NI)
    ).copy()
    iota_c = pids.astype(np.float32).reshape(128, 1)
    offs = ((pids % 16) * NI).astype(np.float32).reshape(128, 1)
    e8 = (pids[None, :] // 16 == np.arange(BSH)[:, None]).astype(np.float32)
    i2 = np.eye(2, dtype=np.float32)
    sel01 = np.zeros((2, 2, 128), dtype=np.float32)
    sel01[0, 0, :] = 1.0
    sel01[1, 1, :] = 1.0
    e2 = (pids == 2).astype(np.float32).reshape(128, 1)
    ones1 = np.ones((1, 128), dtype=np.float32)
    den2 = np.full((2, C), -1e30, dtype=np.float32)
    den2[0, :] = den_scores[: L + 3]
    den2[1, : L + 1] = den_scores[L + 3 :]

    tmask = np.arange(T)[None, :] < input_lens[:, None]   # [B, T] valid

    in_maps = []
    for k in range(NCORES):
        sl = slice(k * BSH, (k + 1) * BSH)
        # bt-row p = s*16 + toff holds t = toff*256 + i; lpT column g = i*128 + p
        lp_bt = log_probs[sl].reshape(BSH, 16, NI, C)       # [s, toff, i, c]
        lpt = np.ascontiguousarray(
            lp_bt.transpose(3, 2, 0, 1).reshape(C, BT)      # [c, (i, s, toff)]
        ).astype(bf16)
        lab_bt = labels[sl].reshape(BSH, 16, NI).astype(np.float32)
        lab_bt = np.where(tmask[sl].reshape(BSH, 16, NI), lab_bt, 200.0)
        labt = np.ascontiguousarray(
            lab_bt.transpose(2, 0, 1).reshape(1, BT)
        ).astype(bf16)
        in_maps.append(
            dict(
                lpt=lpt,
                labt=labt,
                lens=input_lens[sl].astype(np.float32).reshape(BSH, 1),
                den2=den2,
                iota_i=iota_i,
                iota_c=iota_c,
                offs=offs,
                e8=e8,
                i2=i2,
                sel01=sel01,
                e2=e2,
                ones1=ones1,
            )
        )
    return in_maps


def _combine_host(results):
    """Fold per-core device outputs into the scalar loss (float64 host fold)."""
    num = 0.0
    logM_all = []  # [64, NCHUNKS_TOTAL, 2, 2] in global sequence order
    fs = None
    corr = LCH * np.log(SCALE)
    for res in results:
        num += float(res["numpart"].sum(dtype=np.float64))
        fs = float(res["fs"][0, 0])
        cl = res["chunklog"].astype(np.float64)  # [128, 4, NCH]
        # partition p -> (seq_local = p//16, toff = p%16); chunk order (toff, c)
        cl = cl.reshape(BSH, 16, 4, NCH)
        cl = np.transpose(cl, (0, 1, 3, 2)).reshape(BSH, 16 * NCH, 2, 2)
        logM_all.append(cl - corr)
    mats = np.concatenate(logM_all, axis=0)  # [64, 512, 2, 2]

    def compose(Bm, Am):
        # C = B o A : C[i,j] = LSE_k(B[i,k] + A[k,j])
        s = Bm[..., :, :, None] + Am[..., None, :, :]  # [..., i, k, j]
        return _lse(s, axis=-2)

    while mats.shape[1] > 1:
        n = mats.shape[1]
        if n % 2:
            last = mats[:, -1:]
            mats = compose(mats[:, 1::2], mats[:, 0:-1:2])
            mats = np.concatenate([mats, last], axis=1)
        else:
            mats = compose(mats[:, 1::2], mats[:, 0::2])
    den = float(mats[:, 0, 0, 0].sum()) + B * fs
    return np.float32(num - den)


def _lse(x, axis):
    m = np.max(x, axis=axis, keepdims=True)
    m = np.where(np.isfinite(m), m, 0.0)
    out = np.squeeze(m, axis) + np.log(
        np.sum(np.exp(x - m), axis=axis)
    )
    return out


def kernel(log_probs, den_scores, input_lens, labels):
    nc = _get_program()
    in_maps = _make_in_maps(
        np.asarray(log_probs), np.asarray(den_scores),
        np.asarray(input_lens), np.asarray(labels),
    )
    res = run_bass_kernel_spmd(nc, in_maps, core_ids=list(range(NCORES)))
    return _combine_host(res.results)


# revision 8
# speedup vs baseline: 2.3690x; 1.5247x over previous
"""CRF loss (2-state FSA) on 8 Trainium2 NeuronCores — transposed-layout v4.

Math: with y = exp(log_probs), the per-step denominator scores are linear in y:
  S0 = sum_c y[c]*U0[c];  S1 = sum_c y[c]*U1[c];  p = y[2]
where U0/U1 are softmax segments of den_scores mapped through the arc table.
The 2-state forward recurrence runs in REAL space as products of 2x2 matrices
  M_t = [[S0, S1], [p*e00, p*e11]]
composed on-device over chunks of L=2 steps (scaled by 32 per step against
underflow; exact correction removed on host). Steps past input_len become
32*I. The host folds per-sequence chunk matrices in log space and sums.

Layout: host ships lp TRANSPOSED as lpT[c, g] fp16 with column order
g = i*128 + pi, where pi = (seq_local*16 + toff) is the scan partition and
i the within-partition step (t = toff*256 + i). Then:
  - exp on ACT gives y16T in the same layout (fp16 keeps the denominator
    bias ~64x below bf16);
  - S0/S1/p for scan column i = one tiny PE matmul y16T[:, i-block].T @ W
    (W columns: U0, U1, onehot(2), 0) straight into scan-layout PSUM;
  - numerator: PE broadcasts labels (ones[1,128].T @ labT) into PSUM
    (software-pipelined one chunk ahead), one fused DVE STT per 1024-chunk
    computes (lab==iota_c)*lpT, and PE ones-matmuls accumulate the masked
    values into a [128,1] PSUM (exact fp16 gather, fp32 accumulation).

All small constants ride in one packed [128, 902] f32 DMA. All Ln's happen
at the tail so the ACT Exp table is loaded exactly once.

Sharding: data-parallel over batch; core k owns sequences [8k, 8k+8).
"""

import os
import sys

import numpy as np

for _p in ("/opt/trn_rl_repo", os.path.expanduser("~/.axon_site/_ro/trn_rl_repo")):
    if os.path.isdir(_p) and _p not in sys.path:
        sys.path.insert(0, _p)

import concourse.bacc as bacc
import concourse.bass as bass
import concourse.mybir as mybir
import concourse.tile as tile
from concourse.bass_utils import run_bass_kernel_spmd

F32 = mybir.dt.float32
FP16 = mybir.dt.float16
Alu = mybir.AluOpType
Act = mybir.ActivationFunctionType

L = 125
C = 128          # symbol classes
B, T = 64, 4096
NCORES = 8
BSH = B // NCORES            # sequences per core = 8
BT = BSH * T                 # positions per core = 32768
NI = BT // 128               # steps per scan partition = 256
NQ = 4                       # quarters
NH = 8                       # half-quarters (DMA/exp granularity)
GH = BT // NH                # positions per half = 4096
NIQ = NI // NQ               # scan steps per quarter = 64
NIH = NI // NH               # scan steps per half = 32
LCH = 2                      # scan chunk length (steps composed on device)
NCH = NI // LCH              # 128 chunk matrices per partition
NCQ = NIQ // LCH             # chunk matrices per quarter = 32
SCALE = 32.0                 # per-step scaling against fp32 underflow
CK = 1024                    # numerator STT chunk positions
NCKH = GH // CK              # chunks per half = 4

# packed-constant column offsets (f32 columns in cpk [128, CPK])
O_IOTAI, O_IOTAC, O_OFFS, O_E2 = 0, NI, NI + 1, NI + 2
O_E8 = NI + 3                # [8, 128]
O_I2 = O_E8 + 128            # [2, 2]
O_SEL0 = O_I2 + 2            # [2, 128]
O_SEL1 = O_SEL0 + 128        # [2, 128]
O_DEN = O_SEL1 + 128         # [2, 128]
O_LENS = O_DEN + 128         # [8, 1]
O_ONES = O_LENS + 1          # [1, 128]
CPK = O_ONES + 128


def _build_program():
    nc = bacc.Bacc("TRN2", target_bir_lowering=False, debug=False)

    lpt_d = nc.dram_tensor("lpt", [128, BT], FP16, kind="ExternalInput")
    labt_d = nc.dram_tensor("labt", [1, BT], FP16, kind="ExternalInput")
    cpk_d = nc.dram_tensor("cpk", [128, CPK], F32, kind="ExternalInput")

    chunklog_d = nc.dram_tensor("chunklog", [128, 4, NCH], F32, kind="ExternalOutput")
    numpart_d = nc.dram_tensor("numpart", [128, 1], F32, kind="ExternalOutput")
    fs_d = nc.dram_tensor("fs", [1, 1], F32, kind="ExternalOutput")

    with tile.TileContext(nc) as tc:
        with (
            tc.tile_pool(name="const", bufs=1) as cpool,
            tc.tile_pool(name="small", bufs=1) as spool,
            tc.tile_pool(name="lpt", bufs=4) as lp_pool,
            tc.tile_pool(name="y16", bufs=4) as y_pool,
            tc.tile_pool(name="msk", bufs=3) as m_pool,
            tc.tile_pool(name="scan", bufs=1) as scan_pool,
            tc.tile_pool(name="psS", bufs=2, space=bass.MemorySpace.PSUM) as psS,
            tc.tile_pool(name="psL", bufs=2, space=bass.MemorySpace.PSUM) as psL,
            tc.tile_pool(name="psN", bufs=1, space=bass.MemorySpace.PSUM) as psN,
            tc.tile_pool(name="psU", bufs=1, space=bass.MemorySpace.PSUM) as psU,
        ):
            # ---------------- packed constants (one DMA) ----------------
            cpk = cpool.tile([128, CPK], F32)
            nc.sync.dma_start(cpk[:], cpk_d.ap())
            labt_sb = cpool.tile([1, BT], FP16)
            nc.sync.dma_start(labt_sb[:], labt_d.ap())

            iota_i = cpk[:, O_IOTAI:O_IOTAI + NI]
            iota_c = cpk[:, O_IOTAC:O_IOTAC + 1]
            offs = cpk[:, O_OFFS:O_OFFS + 1]
            e2f = cpk[:, O_E2:O_E2 + 1]
            e8 = cpk[0:BSH, O_E8:O_E8 + 128]
            i2 = cpk[0:2, O_I2:O_I2 + 2]
            sel0 = cpk[0:2, O_SEL0:O_SEL0 + 128]
            sel1 = cpk[0:2, O_SEL1:O_SEL1 + 128]
            den_sb = cpk[0:2, O_DEN:O_DEN + C]
            lens_sb = cpk[0:BSH, O_LENS:O_LENS + 1]
            onesf = cpk[0:1, O_ONES:O_ONES + 128]

            ones1 = cpool.tile([1, 128], FP16)
            nc.vector.tensor_copy(ones1[:], onesf)
            ones128 = cpool.tile([128, 1], FP16)
            nc.gpsimd.memset(ones128[:], 1.0)

            # ---------------- arc weights (u = per-state softmax) ----------------
            ed = spool.tile([2, C], F32)
            nc.scalar.activation(ed[:], den_sb, Act.Exp)
            zd = spool.tile([2, 1], F32)
            nc.vector.tensor_reduce(zd[:], ed[:], mybir.AxisListType.X, Alu.add)
            rd = spool.tile([2, 1], F32)
            nc.vector.reciprocal(rd[:], zd[:])
            u = spool.tile([2, C], F32)
            nc.vector.tensor_scalar(u[:], ed[:], rd[:, 0:1], None, Alu.mult)

            # urows[0, c] = U0 weight of class c, urows[1, c] = U1 weight
            urows = spool.tile([2, C], F32)
            nc.gpsimd.memset(urows[:], 0.0)
            nc.vector.tensor_copy(urows[0:1, 1:2], u[0:1, 0:1])
            nc.vector.tensor_copy(urows[0:2, 3:128], u[0:2, 1:126])
            u_ps = psU.tile([128, 2], F32, tag="init")
            nc.tensor.matmul(u_ps[:], urows[:], i2, start=True, stop=True)
            # moving operand W: [U0, U1, onehot(2), 0]
            w4 = spool.tile([128, 4], FP16)
            nc.gpsimd.memset(w4[:], 0.0)
            nc.vector.tensor_copy(w4[:, 0:2], u_ps[:])
            nc.vector.tensor_copy(w4[:, 2:3], e2f)

            # e_b[:, 0] = e00 = u0[126] bcast, e_b[:, 1] = e11 = u1[0] bcast
            e_ps = psU.tile([128, 2], F32, tag="init")
            nc.tensor.matmul(
                e_ps[:, 0:1], sel0, u[0:2, 126:127], start=True, stop=True
            )
            nc.tensor.matmul(
                e_ps[:, 1:2], sel1, u[0:2, 0:1], start=True, stop=True
            )
            e_b = spool.tile([128, 2], F32)
            nc.vector.tensor_copy(e_b[:], e_ps[:])

            # ---------------- per-partition length mask ----------------
            thr_ps = psU.tile([128, 1], F32, tag="init")
            nc.tensor.matmul(thr_ps[:], e8, lens_sb, start=True, stop=True)
            thr = spool.tile([128, 1], F32)
            nc.vector.tensor_tensor(thr[:], thr_ps[:], offs, Alu.subtract)
            m01 = spool.tile([128, NI], F32)
            nc.vector.tensor_scalar(m01[:], iota_i, thr[:, 0:1], None, Alu.is_lt)
            w32 = spool.tile([128, NI], F32)   # 32*m
            nc.vector.tensor_scalar(w32[:], m01[:], SCALE, None, Alu.mult)
            w32c = spool.tile([128, NI], F32)  # 32*(1-m)
            nc.vector.tensor_scalar(w32c[:], m01[:], -SCALE, SCALE, Alu.mult, Alu.add)

            # ---------------- persistent buffers ----------------
            # v_sb[pi, 4*i + n]: n=0 S0, n=1 S1, n=2 p
            v_sb = spool.tile([128, 4 * NI], F32)
            sm0 = spool.tile([128, NI], F32)
            sm1 = spool.tile([128, NI], F32)
            a_t = spool.tile([128, NI], F32)
            b_t = spool.tile([128, NI], F32)
            # composed 2x2 chunk entries, [128, (e, NCH)] for e in 00,01,10,11
            plogbuf = spool.tile([128, 4 * NCH], F32)
            plogln = spool.tile([128, 4 * NCH], F32)

            numacc = psN.tile([128, 1], F32)
            n_mm = 0
            N_MM_TOTAL = NH * NCKH * (CK // 128)

            def bcast(hh, h):
                lab_ps = psL.tile([128, CK], F32)
                for z in range(CK // 512):
                    nc.tensor.matmul(
                        lab_ps[:, z * 512 : (z + 1) * 512],
                        ones1[:],
                        labt_sb[:, hh * GH + h * CK + z * 512 :
                                hh * GH + h * CK + (z + 1) * 512],
                        start=True, stop=True,
                    )
                return lab_ps

            # ---------------- streaming main loop (half-quarter granularity) --
            lab_q = []
            for hh in range(NH):
                lpt = lp_pool.tile([128, GH], FP16)
                nc.sync.dma_start(lpt[:], lpt_d.ap()[:, hh * GH : (hh + 1) * GH])

                # ---- numerator: PE bcast (1 chunk ahead) -> STT -> PE reduce
                if hh == 0:
                    lab_q.append(bcast(0, 0))
                for h in range(NCKH):
                    if h + 1 < NCKH:
                        lab_q.append(bcast(hh, h + 1))
                    elif hh + 1 < NH:
                        lab_q.append(bcast(hh + 1, 0))
                    lab_ps = lab_q.pop(0)
                    msk = m_pool.tile([128, CK], FP16)
                    nc.vector.scalar_tensor_tensor(
                        msk[:],
                        lab_ps[:],
                        iota_c,
                        lpt[:, h * CK : (h + 1) * CK],
                        Alu.is_equal,
                        Alu.mult,
                    )
                    for z in range(CK // 128):
                        nc.tensor.matmul(
                            numacc[:],
                            msk[:, z * 128 : (z + 1) * 128],
                            ones128[:],
                            start=(n_mm == 0), stop=(n_mm == N_MM_TOTAL - 1),
                            skip_group_check=True,
                        )
                        n_mm += 1

                # ---- denominator: exp, S-matmuls into scan-layout PSUM ----
                y16 = y_pool.tile([128, GH], FP16)
                nc.scalar.activation(y16[:], lpt[:], Act.Exp)
                s_ps = psS.tile([128, 4 * NIH], F32)
                for j in range(NIH):
                    nc.tensor.matmul(
                        s_ps[:, 4 * j : 4 * j + 4],
                        y16[:, j * 128 : (j + 1) * 128],
                        w4[:],
                        start=True, stop=True,
                    )
                nc.scalar.copy(
                    v_sb[:, hh * 4 * NIH : (hh + 1) * 4 * NIH], s_ps[:]
                )

                if hh % 2 == 0:
                    continue
                # ---- per-quarter scan prep + chunk compose (Pool) ----
                q = hh // 2
                sl = slice(q * NIQ, (q + 1) * NIQ)
                v3 = v_sb[:].rearrange("p (i n) -> p i n", n=4)[:, sl, :]
                w32q, w32cq = w32[:, sl], w32c[:, sl]
                nc.gpsimd.tensor_tensor(sm0[:, sl], v3[:, :, 0:1], w32q, Alu.mult)
                nc.gpsimd.tensor_tensor(sm0[:, sl], sm0[:, sl], w32cq, Alu.add)
                nc.gpsimd.tensor_tensor(sm1[:, sl], v3[:, :, 1:2], w32q, Alu.mult)
                nc.gpsimd.tensor_scalar(a_t[:, sl], v3[:, :, 2:3], e_b[:, 0:1], None, Alu.mult)
                nc.gpsimd.tensor_tensor(a_t[:, sl], a_t[:, sl], w32q, Alu.mult)
                nc.gpsimd.tensor_scalar(b_t[:, sl], v3[:, :, 2:3], e_b[:, 1:2], None, Alu.mult)
                nc.gpsimd.tensor_tensor(b_t[:, sl], b_t[:, sl], w32q, Alu.mult)
                nc.gpsimd.tensor_tensor(b_t[:, sl], b_t[:, sl], w32cq, Alu.add)

                def tslice(tl, t):
                    return tl[:, sl].rearrange("p (c l) -> p c l", l=LCH)[:, :, t : t + 1]

                # chunk compose (LCH=2): P = M(t1) o M(t0), M(t0) read in place
                qc = slice(q * NCQ, (q + 1) * NCQ)
                pb = plogbuf[:].rearrange("p (e c) -> p e c", e=4)
                s0t, s1t = tslice(sm0, 1), tslice(sm1, 1)
                att, btt = tslice(a_t, 1), tslice(b_t, 1)
                for ci, col in enumerate(("0", "1")):
                    pc0 = tslice(sm0 if col == "0" else sm1, 0)
                    pc1 = tslice(a_t if col == "0" else b_t, 0)
                    o0 = pb[:, 0 + ci, qc]        # e for "0"+col
                    o1 = pb[:, 2 + ci, qc]        # e for "1"+col
                    t1 = scan_pool.tile([128, NCQ], F32, tag=f"t1{col}")
                    nc.gpsimd.tensor_tensor(t1[:], s0t, pc0, Alu.mult)
                    nc.gpsimd.tensor_tensor(o0, s1t, pc1, Alu.mult)
                    nc.gpsimd.tensor_tensor(o0, t1[:], o0, Alu.add)
                    t2 = scan_pool.tile([128, NCQ], F32, tag=f"t2{col}")
                    nc.gpsimd.tensor_tensor(t2[:], att, pc0, Alu.mult)
                    nc.gpsimd.tensor_tensor(o1, btt, pc1, Alu.mult)
                    nc.gpsimd.tensor_tensor(o1, t2[:], o1, Alu.add)

            # ---------------- tail: logs, outputs ----------------
            numpart = spool.tile([128, 1], F32)
            nc.vector.tensor_copy(numpart[:], numacc[:])
            nc.sync.dma_start(numpart_d.ap(), numpart[:])

            nc.scalar.activation(plogln[:], plogbuf[:], Act.Ln)
            nc.sync.dma_start(
                chunklog_d.ap(),
                plogln[:].rearrange("p (e c) -> p e c", e=4),
            )
            fs_sb = spool.tile([1, 1], F32)
            nc.scalar.activation(fs_sb[:], u[0:1, 127:128], Act.Ln)
            nc.sync.dma_start(fs_d.ap(), fs_sb[:])

    nc.compile()
    return nc


_NC_CACHE = None


def _get_program():
    global _NC_CACHE
    if _NC_CACHE is None:
        _NC_CACHE = _build_program()
    return _NC_CACHE


def _make_in_maps(log_probs, den_scores, input_lens, labels):
    fp16 = np.float16
    pids = np.arange(128)
    cpk = np.zeros((128, CPK), dtype=np.float32)
    cpk[:, O_IOTAI:O_IOTAI + NI] = np.arange(NI, dtype=np.float32)[None, :]
    cpk[:, O_IOTAC] = pids
    cpk[:, O_OFFS] = (pids % 16) * NI
    cpk[:, O_E2] = (pids == 2)
    cpk[0:BSH, O_E8:O_E8 + 128] = (pids[None, :] // 16 == np.arange(BSH)[:, None])
    cpk[0:2, O_I2:O_I2 + 2] = np.eye(2)
    cpk[0, O_SEL0:O_SEL0 + 128] = 1.0
    cpk[1, O_SEL1:O_SEL1 + 128] = 1.0
    cpk[0, O_DEN:O_DEN + C] = den_scores[: L + 3]
    cpk[1, O_DEN:O_DEN + L + 1] = den_scores[L + 3 :]
    cpk[1, O_DEN + L + 1:O_DEN + C] = -1e30
    cpk[0, O_ONES:O_ONES + 128] = 1.0

    tmask = np.arange(T)[None, :] < input_lens[:, None]   # [B, T] valid

    in_maps = []
    for k in range(NCORES):
        sl = slice(k * BSH, (k + 1) * BSH)
        cpk_k = cpk.copy()
        cpk_k[0:BSH, O_LENS] = input_lens[sl]
        # bt-row p = s*16 + toff holds t = toff*256 + i; lpT column g = i*128 + p
        lp_bt = log_probs[sl].reshape(BSH, 16, NI, C)       # [s, toff, i, c]
        lpt = np.ascontiguousarray(
            lp_bt.transpose(3, 2, 0, 1).reshape(C, BT)      # [c, (i, s, toff)]
        ).astype(fp16)
        lab_bt = labels[sl].reshape(BSH, 16, NI).astype(np.float32)
        lab_bt = np.where(tmask[sl].reshape(BSH, 16, NI), lab_bt, 200.0)
        labt = np.ascontiguousarray(
            lab_bt.transpose(2, 0, 1).reshape(1, BT)
        ).astype(fp16)
        in_maps.append(dict(lpt=lpt, labt=labt, cpk=cpk_k))
    return in_maps


def _combine_host(results):
    """Fold per-core device outputs into the scalar loss (float64 host fold)."""
    num = 0.0
    logM_all = []  # [64, NCHUNKS_TOTAL, 2, 2] in global sequence order
    fs = None
    corr = LCH * np.log(SCALE)
    for res in results:
        num += float(res["numpart"].sum(dtype=np.float64))
        fs = float(res["fs"][0, 0])
        cl = res["chunklog"].astype(np.float64)  # [128, 4, NCH]
        # partition p -> (seq_local = p//16, toff = p%16); chunk order (toff, c)
        cl = cl.reshape(BSH, 16, 4, NCH)
        cl = np.transpose(cl, (0, 1, 3, 2)).reshape(BSH, 16 * NCH, 2, 2)
        logM_all.append(cl - corr)
    mats = np.concatenate(logM_all, axis=0)  # [64, 512, 2, 2]

    def compose(Bm, Am):
        # C = B o A : C[i,j] = LSE_k(B[i,k] + A[k,j])
        s = Bm[..., :, :, None] + Am[..., None, :, :]  # [..., i, k, j]
        return _lse(s, axis=-2)

    while mats.shape[1] > 1:
        n = mats.shape[1]
        if n % 2:
            last = mats[:, -1:]
            mats = compose(mats[:, 1::2], mats[:, 0:-1:2])
            mats = np.concatenate([mats, last], axis=1)
        else:
            mats = compose(mats[:, 1::2], mats[:, 0::2])
    den = float(mats[:, 0, 0, 0].sum()) + B * fs
    return np.float32(num - den)


def _lse(x, axis):
    with np.errstate(divide="ignore"):
        m = np.max(x, axis=axis, keepdims=True)
        m = np.where(np.isfinite(m), m, 0.0)
        out = np.squeeze(m, axis) + np.log(
            np.sum(np.exp(x - m), axis=axis)
        )
    return out


def kernel(log_probs, den_scores, input_lens, labels):
    nc = _get_program()
    in_maps = _make_in_maps(
        np.asarray(log_probs), np.asarray(den_scores),
        np.asarray(input_lens), np.asarray(labels),
    )
    res = run_bass_kernel_spmd(nc, in_maps, core_ids=list(range(NCORES)))
    return _combine_host(res.results)
